# revision 23
# baseline (speedup 1.0000x reference)
"""GQA causal attention (RoPE) kernel for 8 TRN2 NeuronCores.

Sharding: core = b*4 + g  (b = batch 0..1, g = head-group 0..3).
Each core handles one batch element, 8 query heads (g*8..g*8+7) and the
2 KV heads (g*2, g*2+1) that serve them, plus the matching row-block of
Wo; per-core outputs are partial sums over the hidden dim that the host
reduces across the 4 groups of each batch.

On-core dataflow (all matmuls bf16 with f32 PSUM accumulation):
  QT = Wq_g.T @ X.T   [1024, 2048]   (feature-on-partition layout)
  KT = Wk_g.T @ X.T   [256, 2048]    + RoPE on QT/KT via a PE rotation
  VT = Wv_g.T @ X.T   -> PE-transposed to V [2048, 256]
  per head, per q-128 chunk: S[q,k] = QT_chunk.T x KT (causal-narrowed),
  additive -1e9 upper-tri mask on the diagonal 128-block, P = exp(S/sqrt(d))
  on ScalarE with fused row-sum (accum_out) -> per-row reciprocal ->
  P normalized in place; P blocks PE-transposed to P^T, ctx^T = V.T @ P^T;
  out_partial = ctx @ Wo_g (row block) accumulated over heads in PSUM.
"""

import os

import numpy as np
import ml_dtypes

import concourse.bass as bass
import concourse.mybir as mybir
import concourse.tile as tile
from concourse import bacc, bass_isa
from concourse.bass_utils import run_bass_kernel_spmd
from contextlib import ExitStack

B, S, H = 2, 2048, 4096
NH, NKV, HD = 32, 8, 128
BASE = 10000.0
N_CORES = 8
GROUPS = 4
NH_L = NH // GROUPS        # 8 local q heads
NKV_L = NKV // GROUPS      # 2 local kv heads
HC = H // 128              # 32 hidden chunks
TC = S // 128              # 16 token chunks
TB = S // 512              # 4 token 512-blocks
OC = H // 512              # 8 output-feature 512-blocks
SCALE = 1.0 / float(np.sqrt(HD))
NEG = -1e9

BF16 = mybir.dt.bfloat16
F32 = mybir.dt.float32
EXP = mybir.ActivationFunctionType.Exp
AX = mybir.AxisListType.X
ADD = mybir.AluOpType.add

_PROG = None
LAST_EXEC_NS = None
LAST_RESULTS = None


def _build():
    nc = bacc.Bacc(None, target_bir_lowering=False, debug=False)
    with tile.TileContext(nc) as tc:
        xt_d = nc.dram_tensor("xt", [128, HC, S], BF16, kind="ExternalInput")
        wq_d = nc.dram_tensor("wq", [NH_L, 128, HC, 128], BF16, kind="ExternalInput")
        wk_d = nc.dram_tensor("wk", [NKV_L, 128, HC, 128], BF16, kind="ExternalInput")
        wv_d = nc.dram_tensor("wv", [NKV_L, 128, HC, 128], BF16, kind="ExternalInput")
        wo_d = nc.dram_tensor("wo", [NH_L, 128, H], BF16, kind="ExternalInput")
        cos_d = nc.dram_tensor("cos", [128, S], BF16, kind="ExternalInput")
        sin_d = nc.dram_tensor("sin", [128, S], BF16, kind="ExternalInput")
        rt_d = nc.dram_tensor("rt", [128, 128], BF16, kind="ExternalInput")
        tria_d = nc.dram_tensor("tria", [128, 128], BF16, kind="ExternalInput")
        ident_d = nc.dram_tensor("ident", [128, 128], BF16, kind="ExternalInput")
        out_d = nc.dram_tensor("out_p", [S, H], BF16, kind="ExternalOutput")

        with ExitStack() as stk:
            persist = stk.enter_context(tc.tile_pool(name="persist", bufs=1))
            q_all = persist.tile([128, NH_L, S], BF16, name="q_all", tag="q_all")
            k_all = persist.tile([128, NKV_L, S], BF16, name="k_all", tag="k_all")
            v_all = persist.tile([128, TC, NKV_L * 128], BF16, name="v_all", tag="v_all")
            rt_sb = persist.tile([128, 128], BF16, name="rt_sb", tag="rt_sb")
            tria_sb = persist.tile([128, 128], BF16, name="tria_sb", tag="tria_sb")
            ident_sb = persist.tile([128, 128], BF16, name="ident_sb", tag="ident_sb")
            bias0 = persist.tile([128, 1], F32, name="bias0", tag="bias0")

            nc.sync.dma_start(out=rt_sb[:], in_=rt_d[:])
            nc.sync.dma_start(out=tria_sb[:], in_=tria_d[:])
            nc.sync.dma_start(out=ident_sb[:], in_=ident_d[:])
            nc.any.memset(bias0[:], 0.0)

            # one PSUM pool for the whole program:
            #   tag "acc" (5 banks): projection accumulators, attention AV,
            #                        o-proj accumulators
            #   tag "sp"  (3 banks): rot matmuls, QK scores
            ps_pool = stk.enter_context(tc.tile_pool(name="ps", bufs=1, space="PSUM"))

            def acc_tile():
                return ps_pool.tile([128, 512], F32, name="acc", tag="acc", bufs=4)

            def sp_tile(dt=F32):
                # [128, 1024] row tiles (2 banks): two 512 k-blocks share one
                # tile so the exp runs as a single wide ScalarE instruction
                return ps_pool.tile([128, 1024], dt, name="spt", tag="sp", bufs=2)

            # ---------------- projections ----------------
            # Software-pipelined one projection deep: projection f's
            # post-processing (RoPE rotation matmuls / V transpose) is
            # emitted AFTER projection f+1's matmuls, so the in-order PE
            # queue never waits on the PSUM->SBUF copies feeding the rot
            # matmuls.
            with ExitStack() as proj:
                cs_pool = proj.enter_context(tc.tile_pool(name="csp", bufs=1))
                cos_sb = cs_pool.tile([128, S], BF16, name="cos_sb", tag="cos_sb")
                sin_sb = cs_pool.tile([128, S], BF16, name="sin_sb", tag="sin_sb")
                wpool = proj.enter_context(tc.tile_pool(name="wpool", bufs=5))
                raw_pool = proj.enter_context(tc.tile_pool(name="rawp", bufs=2))
                tmp_pool = proj.enter_context(tc.tile_pool(name="tmpp", bufs=1))

                def load_w_quarters(w_d, f, lo=0, hi=4):
                    ws = []
                    for qtr in range(lo, hi):
                        wt = wpool.tile([128, 8, 128], BF16, name="wt", tag="wt")
                        nc.sync.dma_start(out=wt[:], in_=w_d[f, :, qtr * 8:(qtr + 1) * 8, :])
                        ws.append(wt)
                    return ws

                # minimal first-matmul prefetch: first weight quarter and
                # first xt chunk, then cos/sin + the rest
                ws_v0 = load_w_quarters(wv_d, 0, lo=0, hi=1)
                xt_pool = proj.enter_context(tc.tile_pool(name="xtp", bufs=1))
                xt_first = []
                for i in range(2):
                    t = xt_pool.tile([128, 1, S], BF16, name=f"xtf{i}", tag=f"xtf{i}")
                    nc.sync.dma_start(out=t[:], in_=xt_d[:, i:i + 1, :])
                    xt_first.append(t)
                ws_v0 += load_w_quarters(wv_d, 0, lo=1, hi=4)
                nc.sync.dma_start(out=cos_sb[:], in_=cos_d[:])
                nc.sync.dma_start(out=sin_sb[:], in_=sin_d[:])
                xts = []
                for i in range(1, 16):
                    t = xt_pool.tile([128, 2, S], BF16, name=f"xtt{i}", tag=f"xtt{i}")
                    nc.sync.dma_start(out=t[:], in_=xt_d[:, i * 2:(i + 1) * 2, :])
                    xts.append(t)

                def xt_ap(hc, lo, hi):
                    if hc < 2:
                        return xt_first[hc][:, 0, lo:hi]
                    return xts[hc // 2 - 1][:, hc % 2, lo:hi]

                def project_T(w_d, f, ws=None):
                    if ws is None:
                        ws = load_w_quarters(w_d, f)
                    pss = [acc_tile() for _ in range(TB)]
                    for hc in range(HC):
                        lhsT = ws[hc // 8][:, hc % 8, :]
                        for tb in range(TB):
                            nc.tensor.matmul(
                                pss[tb][:], lhsT, xt_ap(hc, tb * 512, (tb + 1) * 512),
                                start=(hc == 0), stop=(hc == HC - 1),
                            )
                    return pss

                def evac_raw(pss):
                    # PSUM -> SBUF bf16; tb0 on DVE so the acc buf the next
                    # projection's first matmul needs frees fastest
                    raw = raw_pool.tile([128, S], BF16, name="raw", tag="raw")
                    nc.vector.tensor_copy(raw[:, 0:512], pss[0][:])
                    for tb in range(1, TB):
                        nc.scalar.copy(raw[:, tb * 512:(tb + 1) * 512], pss[tb][:])
                    return raw

                def rope_finish(raw, dst, idx):
                    for tb in range(TB):
                        sl = slice(tb * 512, (tb + 1) * 512)
                        rps = sp_tile()
                        nc.tensor.matmul(rps[:, :512], rt_sb[:], raw[:, sl], start=True, stop=True)
                        t1 = tmp_pool.tile([128, 512], F32, name="t1", tag="t1")
                        t2 = tmp_pool.tile([128, 512], F32, name="t2", tag="t2")
                        nc.vector.tensor_mul(t1[:], raw[:, sl], cos_sb[:, sl])
                        nc.vector.tensor_mul(t2[:], rps[:, :512], sin_sb[:, sl])
                        nc.vector.tensor_add(dst[:, idx, sl], t1[:], t2[:])

                def v_finish(raw, f):
                    nc.sync.dma_start_transpose(
                        out=v_all[:, :, f * 128:(f + 1) * 128], in_=raw[:],
                    )

                jobs = (
                    [(wv_d, f, v_finish, (f,)) for f in range(NKV_L)]
                    + [(wk_d, f, rope_finish, (k_all, f)) for f in range(NKV_L)]
                    + [(wq_d, f, rope_finish, (q_all, f)) for f in range(NH_L)]
                )
                pending = None
                for j, (w_d, f, fin, arg) in enumerate(jobs):
                    pss = project_T(w_d, f, ws_v0 if j == 0 else None)
                    if pending is not None:
                        pfin, praw, parg = pending
                        pfin(praw, *parg)
                    raw = evac_raw(pss)
                    pending = (fin, raw, arg)
                pfin, praw, parg = pending
                pfin(praw, *parg)

            # ---------------- attention + output projection ----------------
            with ExitStack() as att:
                wo_pool = att.enter_context(tc.tile_pool(name="wop", bufs=1))
                wo_sb = wo_pool.tile([128, NH_L, H], BF16, name="wo_sb", tag="wo_sb")
                for h in range(NH_L):
                    # SWDGE queues: keep the HWDGE queues free for the
                    # latency-critical P^T transposes
                    nc.gpsimd.dma_start(out=wo_sb[:, h, :], in_=wo_d[h])

                ct_pool = att.enter_context(tc.tile_pool(name="ctp", bufs=3))
                osb_pool = att.enter_context(tc.tile_pool(name="osbp", bufs=2))

                ptt_pool = att.enter_context(tc.tile_pool(name="pttp", bufs=3))
                d2_pool = att.enter_context(tc.tile_pool(name="d2p", bufs=2))
                rq_pool = att.enter_context(tc.tile_pool(name="rqp", bufs=3))

                cts_by_qb = {}

                def softmax_part(qb, h):
                    """K-major scores: S^T = K_chunk^T @ Q, mask, exp straight
                    into the AV-ready [k-part, kc, q] layout (no transposes).
                    Denominators via DVE kc-sum + GpSimd partition-sum, off the
                    AV critical path."""
                    kv = h // (NH_L // NKV_L)
                    nkc = 4 * (qb + 1)
                    qlo = qb * 512
                    ptt = ptt_pool.tile([128, 16, 512], BF16, name="ptt", tag="ptt")
                    # full k-chunks two at a time: one 2-bank PSUM tile, one
                    # wide exp
                    for kb2 in range(0, 4 * qb, 2):
                        sp = sp_tile()
                        w = 0
                        for j in (0, 1):
                            kb = kb2 + j
                            if kb >= 4 * qb:
                                break
                            nc.tensor.matmul(
                                sp[:, j * 512:(j + 1) * 512],
                                k_all[:, kv, kb * 128:(kb + 1) * 128],
                                q_all[:, h, qlo:qlo + 512],
                                start=True, stop=True,
                            )
                            w += 512
                        nc.scalar.activation(
                            ptt[:, kb2:kb2 + w // 512, :], sp[:, :w], EXP,
                            bias=bias0[:], scale=SCALE,
                        )
                    # diagonal k-chunks (two per PSUM tile); chunk j covers
                    # q >= j*128, in-chunk diagonal gets the tria^T mask
                    for j2 in (0, 2):
                        sp = sp_tile()
                        for j in (j2, j2 + 1):
                            kb = 4 * qb + j
                            off = (j - j2) * 512
                            nc.tensor.matmul(
                                sp[:, off + j * 128:off + 512],
                                k_all[:, kv, kb * 128:(kb + 1) * 128],
                                q_all[:, h, qlo + j * 128:qlo + 512],
                                start=True, stop=False,
                            )
                            nc.tensor.matmul(
                                sp[:, off + j * 128:off + (j + 1) * 128],
                                tria_sb[:], ident_sb[:],
                                start=False, stop=True,
                            )
                            nc.scalar.activation(
                                ptt[:, kb, j * 128:512],
                                sp[:, off + j * 128:off + 512], EXP,
                                bias=bias0[:], scale=SCALE,
                            )
                            if j:
                                nc.vector.memset(ptt[:, kb, 0:j * 128], 0.0)
                    # denominators: d[q] = sum over (kc, k-part); reciprocal
                    # consumed at AV-evacuation time (a whole block later)
                    d2 = d2_pool.tile([128, 512], F32, name="d2", tag="d2")
                    nc.vector.tensor_reduce(
                        d2[:], ptt[:, 0:nkc, :].transpose([0, 2, 1]),
                        axis=AX, op=ADD)
                    dall = d2_pool.tile([128, 512], F32, name="dall", tag="dall")
                    nc.gpsimd.partition_all_reduce(
                        dall[:], d2[:], 128, bass_isa.ReduceOp.add)
                    rq = rq_pool.tile([128, 512], F32, name="rq", tag="rq")
                    nc.vector.reciprocal(rq[:], dall[:])
                    return ptt, rq

                def av_part(qb, h, ptt, rq):
                    kv = h // (NH_L // NKV_L)
                    nkc = 4 * (qb + 1)
                    av = acc_tile()
                    for kc in range(nkc):
                        d = max(0, kc - 4 * qb)
                        off = d * 128
                        nc.tensor.matmul(
                            av[:, off:512],
                            v_all[:, kc, kv * 128:(kv + 1) * 128],
                            ptt[:, kc, off:512],
                            start=(kc == 0), stop=(kc == nkc - 1),
                        )
                    nc.vector.tensor_mul(cts_by_qb[qb][:, h, :], av[:], rq[:])

                def oproj_quad(qb, qcl, ohalf):
                    # 4 oc-groups of one query row-block, one batched out DMA
                    cts = cts_by_qb[qb]
                    qc = qb * 4 + qcl
                    osb = osb_pool.tile([128, 2048], BF16, name="osb", tag="osb")
                    for i in range(4):
                        oc = ohalf * 4 + i
                        op = acc_tile()
                        for h in range(NH_L):
                            nc.tensor.matmul(
                                op[:],
                                cts[:, h, qcl * 128:(qcl + 1) * 128],
                                wo_sb[:, h, oc * 512:(oc + 1) * 512],
                                start=(h == 0), stop=(h == NH_L - 1),
                            )
                        nc.vector.tensor_copy(osb[:, i * 512:(i + 1) * 512], op[:])
                    nc.gpsimd.dma_start(
                        out=out_d[qc * 128:(qc + 1) * 128,
                                  ohalf * 2048:(ohalf + 1) * 2048],
                        in_=osb[:],
                    )

                # Software-pipelined: the next head's QK matmuls are emitted
                # ahead of this head's AV in the PE stream, so the PE never
                # waits for the exp -> normalize -> transpose chain.  The
                # previous block's o-projection groups are drip-fed 4 per
                # head iteration, so every sparse QK/AV iteration carries
                # ~7us of dense PE work and the HAM clock gate stays warm.
                pairs = [(qb, h) for qb in (3, 2, 1, 0) for h in range(NH_L)]
                oproj_queue = []

                def drip(n):
                    for _ in range(min(n, len(oproj_queue))):
                        dqb, dqcl, dohalf = oproj_queue.pop(0)
                        oproj_quad(dqb, dqcl, dohalf)
                        if not oproj_queue or oproj_queue[0][0] != dqb:
                            cts_by_qb.pop(dqb, None)

                # 2-deep: AV lags QK by two heads, so the exp -> normalize ->
                # P^T-transpose chain has two full iterations of PE work to
                # complete before the PE needs its result.
                DEPTH = 2
                ptts = {}
                for i, (qb, h) in enumerate(pairs):
                    if h == 0:
                        cts_by_qb[qb] = ct_pool.tile(
                            [128, NH_L, 512], BF16, name="cts", tag="ct")
                    ptts[i] = softmax_part(qb, h)
                    j = i - DEPTH
                    if j >= 0:
                        jqb, jh = pairs[j]
                        av_part(jqb, jh, *ptts.pop(j))
                        if jh == NH_L - 1:
                            oproj_queue.extend(
                                (jqb, qcl, oh) for qcl in range(4) for oh in range(2))
                        drip(1)
                for j in range(len(pairs) - DEPTH, len(pairs)):
                    jqb, jh = pairs[j]
                    av_part(jqb, jh, *ptts.pop(j))
                    if jh == NH_L - 1:
                        oproj_queue.extend(
                            (jqb, qcl, oh) for qcl in range(4) for oh in range(2))
                    drip(1)
                drip(len(oproj_queue))
    nc.compile()
    return nc


def _prep_inputs(hidden_states, position_ids, Wq, Wk, Wv, Wo):
    bf = ml_dtypes.bfloat16
    hidden_states = np.asarray(hidden_states, dtype=np.float32)
    position_ids = np.asarray(position_ids)
    Wq = np.asarray(Wq, dtype=np.float32)
    Wk = np.asarray(Wk, dtype=np.float32)
    Wv = np.asarray(Wv, dtype=np.float32)
    Wo = np.asarray(Wo, dtype=np.float32)

    inv_freq = (1.0 / (BASE ** (np.arange(0, HD, 2, dtype=np.float32) / HD))).astype(np.float32)
    rt = np.zeros((128, 128), dtype=np.float32)
    rt[np.arange(64, 128), np.arange(0, 64)] = -1.0
    rt[np.arange(0, 64), np.arange(64, 128)] = 1.0
    rt = rt.astype(bf)
    ident = np.eye(128, dtype=np.float32).astype(bf)
    ii = np.arange(128)
    tria = np.where(ii[None, :] > ii[:, None], np.float32(NEG), np.float32(0.0)).astype(bf)

    per_batch = []
    for b in range(B):
        xt = np.ascontiguousarray(
            hidden_states[b].T.reshape(HC, 128, S).transpose(1, 0, 2)
        ).astype(bf)
        pos = position_ids[b].astype(np.float32)
        freqs = pos[:, None] * inv_freq[None, :]           # [S, 64]
        emb = np.concatenate([freqs, freqs], axis=1)       # [S, 128]
        cos = np.ascontiguousarray(np.cos(emb).T).astype(bf)
        sin = np.ascontiguousarray(np.sin(emb).T).astype(bf)
        per_batch.append((xt, cos, sin))

    in_maps = []
    for core in range(N_CORES):
        b, g = core // GROUPS, core % GROUPS
        xt, cos, sin = per_batch[b]
        wq = np.ascontiguousarray(
            Wq[:, g * NH_L * HD:(g + 1) * NH_L * HD]
            .reshape(HC, 128, NH_L, 128).transpose(2, 1, 0, 3)
        ).astype(bf)
        wk = np.ascontiguousarray(
            Wk[:, g * NKV_L * HD:(g + 1) * NKV_L * HD]
            .reshape(HC, 128, NKV_L, 128).transpose(2, 1, 0, 3)
        ).astype(bf)
        wv = np.ascontiguousarray(
            Wv[:, g * NKV_L * HD:(g + 1) * NKV_L * HD]
            .reshape(HC, 128, NKV_L, 128).transpose(2, 1, 0, 3)
        ).astype(bf)
        wo = np.ascontiguousarray(
            Wo[g * NH_L * HD:(g + 1) * NH_L * HD, :].reshape(NH_L, 128, H)
        ).astype(bf)
        in_maps.append({
            "xt": xt, "wq": wq, "wk": wk, "wv": wv, "wo": wo,
            "cos": cos, "sin": sin, "rt": rt, "tria": tria, "ident": ident,
        })
    return in_maps


def kernel(hidden_states, position_ids, Wq, Wk, Wv, Wo):
    global _PROG, LAST_EXEC_NS, LAST_RESULTS
    if _PROG is None:
        _PROG = _build()
    nc = _PROG
    in_maps = _prep_inputs(hidden_states, position_ids, Wq, Wk, Wv, Wo)
    trace = os.environ.get("BASS_KERNEL_TRACE", "0") == "1"
    res = run_bass_kernel_spmd(nc, in_maps, core_ids=list(range(N_CORES)), trace=trace)
    LAST_EXEC_NS = res.exec_time_ns
    LAST_RESULTS = res
    out = np.zeros((B, S, H), dtype=np.float32)
    for core in range(N_CORES):
        out[core // GROUPS] += res.results[core]["out_p"].astype(np.float32)
    return out

